# revision 10
# baseline (speedup 1.0000x reference)
"""Trainium2 Bass kernel for ColumnParallelLinearWithTopping.

Computes  y[t] = x[t] @ (W_base.T + DeltaW[j] + A[j] @ B[j]),  j = weight_indices[t]

Strategy (8-core adapter-parallel, host-premerged weights, bf16 data path):
  * Host: W_eff[a] = W_base.T + DeltaW[a] + A[a] @ B[a]  (f32 -> bf16),
    tokens stable-sorted by adapter.  Core a gets adapter a's tokens
    (padded to T_PAD) and the full [4096, 4096] W_eff[a].
  * Device (per core): x^T resident in SBUF ([128, KT, T_PAD] bf16, ~9 MB).
    Stream W_eff once (32 col-blocks x 1 MB), mapping:
        psum[col128, tok] += W_tile[k, col128].T @ x^T[k, tok]
    fp32 PSUM accumulation over the 32 k-tiles; moving chunks of 512 tokens
    (one PSUM bank each, double-buffered across col-blocks).
    y^T written back as bf16 [32, 128, T_PAD].
  * Host: transpose y^T shards back and undo the token permutation.
"""
from contextlib import ExitStack

import numpy as np
import ml_dtypes

import concourse.bass as bass
import concourse.mybir as mybir
import concourse.tile as tile
from concourse import bacc
from concourse.bass_utils import run_bass_kernel_spmd

T, D_IN, D_OUT = 8192, 4096, 4096
N_ADAPT, RANK = 8, 16
N_CORES = 8
P = 128
KT = D_IN // P                    # 32 contraction tiles
CB = D_OUT // P                   # 32 column blocks
F32 = mybir.dt.float32
BF16 = mybir.dt.bfloat16
NPBF16 = ml_dtypes.bfloat16

_build_cache: dict = {}
_last_in_maps = None


def _chunk_widths(t_pad: int):
    """Split t_pad into <=512-wide chunks (PSUM bank limit), as evenly as
    possible in multiples of 16 so no matmul stream is too short to hide
    its LDWEIGHTS."""
    n = -(-t_pad // 512)
    widths = []
    rem = t_pad
    for i in range(n, 0, -1):
        w = min(512, ((rem + i - 1) // i + 15) & ~15, rem)
        widths.append(w)
        rem -= w
    assert sum(widths) == t_pad and all(w <= 512 for w in widths)
    return widths


def _build(t_pad: int):
    """Build + compile the SPMD program for per-core token count t_pad."""
    widths = _chunk_widths(t_pad)
    chunks = []
    t0 = 0
    for w in widths:
        chunks.append((t0, w))
        t0 += w
    assert len(chunks) <= 4, f"t_pad={t_pad} needs >4 PSUM chunks"

    nc = bacc.Bacc("TRN2", target_bir_lowering=False, debug=False)
    xk = nc.dram_tensor("xk", [P, KT, t_pad], BF16, kind="ExternalInput").ap()
    wk = nc.dram_tensor("wk", [CB, P, KT, P], BF16, kind="ExternalInput").ap()
    y = nc.dram_tensor("y", [CB, P, t_pad], BF16, kind="ExternalOutput").ap()

    with tile.TileContext(nc) as tc, ExitStack() as ctx:
        xpool = ctx.enter_context(tc.tile_pool(name="xp", bufs=1))
        wpool = ctx.enter_context(tc.tile_pool(name="wp", bufs=3))
        ypool = ctx.enter_context(tc.tile_pool(name="yp", bufs=3))
        pspool = ctx.enter_context(tc.tile_pool(name="ps", bufs=2, space="PSUM"))

        # Startup is x-preload-bandwidth-bound.  All data needed in the first
        # ~40us (x, W for col-blocks 0-2) goes through the ONE sync-queue
        # FIFO in exact PE consumption order, so delivery never races ahead
        # on the wrong tensor; the scalar queue stays empty until cb3+ W
        # prefetch kicks in (deferred naturally by the 3-slot wt pool).
        xt = xpool.tile([P, KT, t_pad], BF16, name="xt")
        wts = [wpool.tile([P, KT, P], BF16, name="wt") for _ in range(3)]
        for kc in range(0, KT, 2):
            # per 2-k group, in exact consumption order of the interleaved
            # (cb0, cb1) matmul pair: w0 piece, x[k], w1 piece, x[k+1]
            nc.sync.dma_start(wts[0][:, kc:kc + 2, :], wk[0, :, kc:kc + 2, :])
            nc.sync.dma_start(xt[:, kc:kc + 1, :], xk[:, kc:kc + 1, :])
            nc.sync.dma_start(wts[1][:, kc:kc + 2, :], wk[1, :, kc:kc + 2, :])
            nc.sync.dma_start(xt[:, kc + 1:kc + 2, :], xk[:, kc + 1:kc + 2, :])
        nc.sync.dma_start(wts[2], wk[2])

        # col-blocks 0+1 interleaved at k granularity: during the x preload
        # window the PE has 2x the work per x k-chunk, so it doesn't outrun
        # the x DMA stream and stall.
        pss = [[pspool.tile([P, 512], F32, name=f"ps{i}", tag=f"ps{i}")
                for i in range(len(chunks))] for _ in range(2)]
        for k in range(KT):
            for cb in range(2):
                for i, (t0, w) in enumerate(chunks):
                    nc.tensor.matmul(
                        pss[cb][i][:, :w], wts[cb][:, k, :], xt[:, k, t0:t0 + w],
                        start=(k == 0), stop=(k == KT - 1),
                    )
        for cb in range(2):
            yt = ypool.tile([P, t_pad], BF16, name="yt")
            for i, (t0, w) in enumerate(chunks):
                nc.vector.tensor_copy(yt[:, t0:t0 + w], pss[cb][i][:, :w])
            nc.sync.dma_start(y[cb], yt)

        for cb in range(2, CB):
            if cb == 2:
                wt = wts[2]
            else:
                wt = wpool.tile([P, KT, P], BF16, name="wt")
                nc.scalar.dma_start(wt, wk[cb])

            psums = [pspool.tile([P, 512], F32, name=f"ps{i}", tag=f"ps{i}")
                     for i in range(len(chunks))]
            yt = ypool.tile([P, t_pad], BF16, name="yt")
            if cb < CB - 1:
                for k in range(KT):
                    lhsT = wt[:, k, :]
                    for i, (t0, w) in enumerate(chunks):
                        nc.tensor.matmul(
                            psums[i][:, :w], lhsT, xt[:, k, t0:t0 + w],
                            start=(k == 0), stop=(k == KT - 1),
                        )
                for i, (t0, w) in enumerate(chunks):
                    nc.vector.tensor_copy(yt[:, t0:t0 + w], psums[i][:, :w])
                nc.sync.dma_start(y[cb], yt)
            else:
                # last col-block: chunk-outer so each chunk's cast + store
                # overlaps the remaining chunks' matmuls (shorter tail)
                for i, (t0, w) in enumerate(chunks):
                    for k in range(KT):
                        nc.tensor.matmul(
                            psums[i][:, :w], wt[:, k, :], xt[:, k, t0:t0 + w],
                            start=(k == 0), stop=(k == KT - 1),
                        )
                    nc.vector.tensor_copy(yt[:, t0:t0 + w], psums[i][:, :w])
                    nc.sync.dma_start(y[cb, :, t0:t0 + w], yt[:, t0:t0 + w])

    nc.compile()
    return nc, t_pad


def kernel(x, weight_indices, W_base, A_buffer, B_buffer, DeltaW):
    global _last_in_maps
    x = np.asarray(x, dtype=np.float32)
    idx = np.asarray(weight_indices).astype(np.int64)
    W_base = np.asarray(W_base, dtype=np.float32)
    A_buffer = np.asarray(A_buffer, dtype=np.float32)
    B_buffer = np.asarray(B_buffer, dtype=np.float32)
    DeltaW = np.asarray(DeltaW, dtype=np.float32)

    order = np.argsort(idx, kind="stable")
    counts = np.bincount(idx, minlength=N_ADAPT)
    cum = np.concatenate([[0], np.cumsum(counts)])
    t_pad = max(64, int(-(-counts.max() // 16)) * 16)

    if t_pad not in _build_cache:
        _build_cache[t_pad] = _build(t_pad)
    nc, _ = _build_cache[t_pad]

    WbT = np.ascontiguousarray(W_base.T)            # [D_IN, D_OUT]
    in_maps = []
    tok_lists = []
    for a in range(N_ADAPT):
        toks = order[cum[a]:cum[a + 1]]
        tok_lists.append(toks)
        Weff = WbT + DeltaW[a] + A_buffer[a] @ B_buffer[a]
        # [cb, p, k, c]: per-colblock contiguous [128, 32*128] DMA chunks
        wkb = np.ascontiguousarray(
            Weff.astype(NPBF16).reshape(KT, P, CB, P).transpose(2, 1, 0, 3))
        xtp = np.zeros((D_IN, t_pad), dtype=np.float32)
        xtp[:, :len(toks)] = x[toks].T
        xkb = np.ascontiguousarray(
            xtp.astype(NPBF16).reshape(KT, P, t_pad).transpose(1, 0, 2))
        in_maps.append({"xk": xkb, "wk": wkb})

    _last_in_maps = in_maps
    res = run_bass_kernel_spmd(nc, in_maps, core_ids=list(range(N_CORES)))

    out = np.empty((T, D_OUT), dtype=np.float32)
    for a in range(N_ADAPT):
        c = len(tok_lists[a])
        yk = np.asarray(res.results[a]["y"]).reshape(D_OUT, t_pad)
        out[tok_lists[a]] = yk[:, :c].T.astype(np.float32)
    return out
